# revision 13
# baseline (speedup 1.0000x reference)
"""Trainium2 Bass kernel for nn_AttnInteractionLayer_2851858284689.

Measured 134166ns HW exec (8 cores, NTFF; rel err 5.4e-3) vs 164808ns for
the session-start baseline under the same (partly throttled) chip clocks.
Progression: 146608 (3-engine attempt reverted; GPSIMD unusable) -> 141838
(2-engine rebalance) -> 135895 (norm split + fused nmr + DMA tweaks) ->
134809 (software-pipelined norms) -> 134225 (act-table prefetch) -> 134166
(bn_aggr replaced by a batched moment combine over contiguous [128,SUB]
planes; bn_stats writes its 6-tuple into columns of a [128,6,SUB] tile).

Math: the reference's mislabeled einsum makes attention collapse to `vals`,
so the module is  out = LayerNorm(leaky_relu(x @ (W_v.reshape(256,512) + W_r)))
(gamma=1, beta=0).

v3 = 2-engine (ACT+DVE) design tuned with REAL per-instruction HW costs
(GPSIMD tensor ops measured 7.6us/subtile on HW and poison concurrent DVE
ops, so the Pool engine is left idle):
  - ACT: grouped-4 Prelu (2341ns), batched sqrt(var+eps), ~4.5/8 of the
    normalizes as Identity(y*rstd - mu*rstd) (962ns each).
  - DVE: bn_stats (674-796ns) + bn_aggr (150ns) per subtile, reciprocal,
    ~3.5/8 of the normalizes as (y-mu)*rstd tensor_scalar (477ns each).
  - PE: bf16 matmuls, p-state warmup dummies so block 0 runs at 2.4GHz.
  - DMA: fully contiguous layouts (4KB/8KB per-partition lines), w split
    per k-chunk behind x0, output written in halves to cut the tail.
"""

import numpy as np
import ml_dtypes

import concourse.bass as bass
import concourse.tile as tile
from concourse import bacc, mybir
from concourse.bass_utils import run_bass_kernel_spmd


def _ensure_ntff_hook():
    """This image lacks ``antenv.axon_hooks``; inject it (ctypes on
    libaxon_pjrt.so) so run_bass_kernel_spmd(trace=True) works."""
    try:
        from antenv.axon_hooks import get_axon_ntff_profile_hook  # noqa: F401
        return
    except ImportError:
        pass
    try:
        import contextlib
        import ctypes
        import sys
        import types

        lib = ctypes.CDLL("/opt/axon/libaxon_pjrt.so")
        if not hasattr(lib, "axon_start_nrt_profile"):
            return
        lib.axon_start_nrt_profile.argtypes = [
            ctypes.POINTER(ctypes.c_int64), ctypes.c_size_t]
        lib.axon_start_nrt_profile.restype = ctypes.c_int64
        lib.axon_stop_nrt_profile.argtypes = [ctypes.c_char_p]
        lib.axon_stop_nrt_profile.restype = ctypes.c_int64

        @contextlib.contextmanager
        def _hook(output_dir, device_ids):
            import jax
            jax.devices()
            if device_ids:
                ids = (ctypes.c_int64 * len(device_ids))(*device_ids)
                rc = lib.axon_start_nrt_profile(ids, len(device_ids))
            else:
                rc = lib.axon_start_nrt_profile(None, 0)
            if rc != 0:
                raise RuntimeError(f"axon_start_nrt_profile rc={rc}")
            try:
                yield
            finally:
                lib.axon_stop_nrt_profile(str(output_dir).encode())

        import antenv
        mod = types.ModuleType("antenv.axon_hooks")
        mod.get_axon_ntff_profile_hook = lambda: _hook
        mod.set_axon_ntff_profile_hook = lambda h: None
        sys.modules["antenv.axon_hooks"] = mod
        antenv.axon_hooks = mod
    except Exception:
        pass


_ensure_ntff_hook()

import os

R, F, IN, OUT = 4096, 32, 256, 512
N_CORES = 8
TOKENS = R * F                   # 131072
TPC = TOKENS // N_CORES          # 16384
KC = IN // 128                   # 2
BLK = 2048
NBLK = TPC // BLK                # 8
SUB = BLK // 128                 # 16
EPS = 1e-5
NEG_SLOPE = 0.01
BF16 = mybir.dt.bfloat16
F32 = mybir.dt.float32
AF = mybir.ActivationFunctionType
ALU = mybir.AluOpType

# normalize engine split: DVE norms per block (rest on ACT).
# DVE fixed load (bn_stats+aggr) >> ACT fixed (prelu), so ACT takes most.
_NDVE_EVEN = int(os.environ.get("KV3_NDVE_EVEN", "9"))
_NDVE_ODD = int(os.environ.get("KV3_NDVE_ODD", "8"))
_NDVE_TAIL = int(os.environ.get("KV3_NDVE_TAIL", "8"))  # last block


def _n_dve(b):
    if b >= NBLK - 1:
        return _NDVE_TAIL
    return _NDVE_ODD if b % 2 else _NDVE_EVEN


_compiled = {}


def _build_nc():
    nc = bacc.Bacc(None)
    xT = nc.declare_dram_parameter("xT", [128, NBLK, KC, BLK], BF16, isOutput=False)
    w = nc.declare_dram_parameter("w", [KC, 128, OUT], BF16, isOutput=False)
    y = nc.declare_dram_parameter("y", [128, NBLK, SUB, OUT], BF16, isOutput=True)

    with tile.TileContext(nc) as tc:
        with (
            tc.tile_pool(name="singles", bufs=1) as singles,
            tc.tile_pool(name="xpool", bufs=4) as xpool,
            tc.tile_pool(name="ypool", bufs=4) as ypool,
            tc.tile_pool(name="opool", bufs=4) as opool,
            tc.tile_pool(name="stats", bufs=3) as stats_pool,
            tc.tile_pool(name="psum", bufs=2, space="PSUM") as psum,
        ):
            wdum = singles.tile([128, 128], BF16)
            nc.gpsimd.memset(wdum, 0.01)
            w_sb = singles.tile([128, KC, OUT], BF16)
            eps_sb = singles.tile([128, 1], F32)
            nc.vector.memset(eps_sb, EPS)
            # act-table prefetch: touch all three functions on a 1-elem tile
            # so the 1.3us table load happens during the x0 DMA, not before
            # the first real Prelu.
            tdum = singles.tile([128, 1], F32)
            nc.scalar.activation(tdum, eps_sb, AF.Prelu, alpha=NEG_SLOPE)
            nc.scalar.activation(tdum, eps_sb, AF.Sqrt)
            nc.scalar.activation(tdum, eps_sb, AF.Identity)

            def emit_norms(b, y_sb, mu, rstd, nmr):
                o_sb = opool.tile([128, SUB, OUT], BF16, name="o_sb")
                nd = _n_dve(b)
                for i in range(nd):
                    nc.vector.tensor_scalar(
                        o_sb[:, i, :], y_sb[:, i, :],
                        scalar1=mu[:, i:i + 1],
                        scalar2=rstd[:, i:i + 1],
                        op0=ALU.subtract, op1=ALU.mult,
                    )
                for i in range(nd, SUB):
                    nc.scalar.activation(
                        o_sb[:, i, :], y_sb[:, i, :], AF.Identity,
                        bias=nmr[:, i:i + 1],
                        scale=rstd[:, i:i + 1],
                    )
                if b == NBLK - 1:
                    for i in range(1, SUB, 2):
                        nc.sync.dma_start(
                            out=y[:, b, i - 1:i + 1], in_=o_sb[:, i - 1:i + 1])
                else:
                    nc.sync.dma_start(out=y[:, b, 0:SUB // 2], in_=o_sb[:, 0:SUB // 2])
                    nc.sync.dma_start(out=y[:, b, SUB // 2:SUB], in_=o_sb[:, SUB // 2:SUB])

            prev = None
            for b in range(NBLK):
                x_sb = xpool.tile([128, KC, BLK], BF16, name="x_sb")
                nc.sync.dma_start(out=x_sb, in_=xT[:, b])
                if b == 0:
                    for c in range(KC):
                        nc.sync.dma_start(out=w_sb[:, c, :], in_=w[c])

                y_sb = ypool.tile([128, SUB, OUT], BF16, name="y_sb")
                st = stats_pool.tile([128, 6, SUB], F32, name="st")

                for g in range(SUB // 4):
                    ps = psum.tile([128, 4, OUT], F32, name="ps")
                    if b == 0 and g == 0:
                        # p-state warmup: keep PE busy from t~0.5us so the
                        # 0.65/1.2GHz ramp is spent on throwaway work.
                        for _ in range(12):
                            nc.tensor.matmul(
                                ps[:, 0, 0:128], lhsT=wdum, rhs=wdum,
                                start=True, stop=True,
                            )
                    for j in range(4):
                        i = g * 4 + j
                        nc.tensor.matmul(
                            ps[:, j, :], lhsT=x_sb[:, 0, bass.ts(i, 128)],
                            rhs=w_sb[:, 0, :], start=True, stop=False,
                        )
                        nc.tensor.matmul(
                            ps[:, j, :], lhsT=x_sb[:, 1, bass.ts(i, 128)],
                            rhs=w_sb[:, 1, :], start=False, stop=True,
                        )
                    if b == 0 and g == 0:
                        nc.scalar.activation(
                            y_sb[:, 0:2, :], ps[:, 0:2, :], AF.Prelu,
                            alpha=NEG_SLOPE,
                        )
                        nc.scalar.activation(
                            y_sb[:, 2:4, :], ps[:, 2:4, :], AF.Prelu,
                            alpha=NEG_SLOPE,
                        )
                    else:
                        nc.scalar.activation(
                            y_sb[:, g * 4:(g + 1) * 4, :], ps, AF.Prelu,
                            alpha=NEG_SLOPE,
                        )
                    for j in range(4):
                        i = g * 4 + j
                        nc.vector.bn_stats(st[:, :, i], y_sb[:, i, :])

                if prev is not None:
                    emit_norms(*prev)

                # combine even/odd bn_stats moments, batched over contiguous
                # [128, SUB] planes (replaces 8 bn_aggr with 7 cheap DVE ops):
                # mu = (me+mo)/2 ; var = (M2e+M2o)/512 + ((me-mo)/2)^2
                me, m2e = st[:, 1, :], st[:, 2, :]
                mo, m2o = st[:, 4, :], st[:, 5, :]
                sm = stats_pool.tile([128, SUB], F32, name="sm")
                dm = stats_pool.tile([128, SUB], F32, name="dm")
                dmh = stats_pool.tile([128, SUB], F32, name="dmh")
                dmq = stats_pool.tile([128, SUB], F32, name="dmq")
                sv = stats_pool.tile([128, SUB], F32, name="sv")
                mu = stats_pool.tile([128, SUB], F32, name="mu")
                var = stats_pool.tile([128, SUB], F32, name="var")
                nc.vector.tensor_tensor(sm, me, mo, ALU.add)
                nc.vector.tensor_scalar_mul(mu, sm, 0.5)
                nc.vector.tensor_tensor(dm, me, mo, ALU.subtract)
                nc.vector.tensor_scalar_mul(dmh, dm, 0.5)
                nc.vector.tensor_tensor(dmq, dmh, dmh, ALU.mult)
                nc.vector.tensor_tensor(sv, m2e, m2o, ALU.add)
                nc.vector.scalar_tensor_tensor(
                    var, sv, 1.0 / OUT, dmq, op0=ALU.mult, op1=ALU.add)

                std = stats_pool.tile([128, SUB], F32, name="std")
                nc.scalar.activation(std, var, AF.Sqrt, bias=eps_sb)
                rstd = stats_pool.tile([128, SUB], F32, name="rstd")
                nc.vector.reciprocal(rstd, std)
                # bias for ACT-normalized subtiles: -mean*rstd (one stt op)
                nmr = stats_pool.tile([128, SUB], F32, name="nmr")
                nc.vector.scalar_tensor_tensor(
                    nmr, mu, -1.0, rstd, op0=ALU.mult, op1=ALU.mult)


                prev = (b, y_sb, mu, rstd, nmr)
            emit_norms(*prev)
    nc.finalize()
    return nc


def _get_nc():
    if "nc" not in _compiled:
        _compiled["nc"] = _build_nc()
    return _compiled["nc"]


def _in_maps(x, W_v, W_r):
    x = np.asarray(x, dtype=np.float32)
    W = (np.asarray(W_v, dtype=np.float32).reshape(IN, OUT)
         + np.asarray(W_r, dtype=np.float32))
    w_dev = np.ascontiguousarray(
        W.reshape(KC, 128, OUT).astype(ml_dtypes.bfloat16))

    xs = x.reshape(TOKENS, IN)
    in_maps = []
    for c in range(N_CORES):
        shard = xs[c * TPC:(c + 1) * TPC]                    # [TPC, IN]
        xt = np.ascontiguousarray(
            shard.reshape(NBLK, BLK, KC, 128).transpose(3, 0, 2, 1)
            .astype(ml_dtypes.bfloat16))                     # [128,NBLK,KC,BLK]
        in_maps.append({"xT": xt, "w": w_dev})
    return in_maps


def _gather(res):
    outs = []
    for c in range(N_CORES):
        yd = np.asarray(res.results[c]["y"])                 # [128,NBLK,SUB,OUT]
        outs.append(yd.astype(np.float32).transpose(1, 2, 0, 3).reshape(TPC, OUT))
    return np.concatenate(outs, axis=0).reshape(R, F, OUT)


def kernel(x, W_q, W_k, W_v, W_r, ln_gamma, ln_beta):
    nc = _get_nc()
    in_maps = _in_maps(x, W_v, W_r)
    res = run_bass_kernel_spmd(nc, in_maps, list(range(N_CORES)))
    out = _gather(res)

    gamma = np.asarray(ln_gamma, dtype=np.float32)
    beta = np.asarray(ln_beta, dtype=np.float32)
    if not (np.all(gamma == 1.0) and np.all(beta == 0.0)):
        out = out * gamma + beta
    return out.astype(np.float32)


# revision 15
# speedup vs baseline: 1.0204x; 1.0204x over previous
"""Trainium2 Bass kernel for nn_AttnInteractionLayer_2851858284689.

Measured 134166ns HW exec (8 cores, NTFF; rel err 5.4e-3) vs 164808ns for
the session-start baseline under the same (partly throttled) chip clocks.
Progression: 146608 (3-engine attempt reverted; GPSIMD unusable) -> 141838
(2-engine rebalance) -> 135895 (norm split + fused nmr + DMA tweaks) ->
134809 (software-pipelined norms) -> 134225 (act-table prefetch) -> 134166
(bn_aggr replaced by a batched moment combine over contiguous [128,SUB]
planes; bn_stats writes its 6-tuple into columns of a [128,6,SUB] tile).

Math: the reference's mislabeled einsum makes attention collapse to `vals`,
so the module is  out = LayerNorm(leaky_relu(x @ (W_v.reshape(256,512) + W_r)))
(gamma=1, beta=0).

v3 = 2-engine (ACT+DVE) design tuned with REAL per-instruction HW costs
(GPSIMD tensor ops measured 7.6us/subtile on HW and poison concurrent DVE
ops, so the Pool engine is left idle):
  - ACT: grouped-4 Prelu (2341ns), batched sqrt(var+eps), ~4.5/8 of the
    normalizes as Identity(y*rstd - mu*rstd) (962ns each).
  - DVE: bn_stats (674-796ns) + bn_aggr (150ns) per subtile, reciprocal,
    ~3.5/8 of the normalizes as (y-mu)*rstd tensor_scalar (477ns each).
  - PE: bf16 matmuls, p-state warmup dummies so block 0 runs at 2.4GHz.
  - DMA: fully contiguous layouts (4KB/8KB per-partition lines), w split
    per k-chunk behind x0, output written in halves to cut the tail.
"""

import numpy as np
import ml_dtypes

import concourse.bass as bass
import concourse.tile as tile
from concourse import bacc, mybir
from concourse.bass_utils import run_bass_kernel_spmd


def _ensure_ntff_hook():
    """This image lacks ``antenv.axon_hooks``; inject it (ctypes on
    libaxon_pjrt.so) so run_bass_kernel_spmd(trace=True) works."""
    try:
        from antenv.axon_hooks import get_axon_ntff_profile_hook  # noqa: F401
        return
    except ImportError:
        pass
    try:
        import contextlib
        import ctypes
        import sys
        import types

        lib = ctypes.CDLL("/opt/axon/libaxon_pjrt.so")
        if not hasattr(lib, "axon_start_nrt_profile"):
            return
        lib.axon_start_nrt_profile.argtypes = [
            ctypes.POINTER(ctypes.c_int64), ctypes.c_size_t]
        lib.axon_start_nrt_profile.restype = ctypes.c_int64
        lib.axon_stop_nrt_profile.argtypes = [ctypes.c_char_p]
        lib.axon_stop_nrt_profile.restype = ctypes.c_int64

        @contextlib.contextmanager
        def _hook(output_dir, device_ids):
            import jax
            jax.devices()
            if device_ids:
                ids = (ctypes.c_int64 * len(device_ids))(*device_ids)
                rc = lib.axon_start_nrt_profile(ids, len(device_ids))
            else:
                rc = lib.axon_start_nrt_profile(None, 0)
            if rc != 0:
                raise RuntimeError(f"axon_start_nrt_profile rc={rc}")
            try:
                yield
            finally:
                lib.axon_stop_nrt_profile(str(output_dir).encode())

        import antenv
        mod = types.ModuleType("antenv.axon_hooks")
        mod.get_axon_ntff_profile_hook = lambda: _hook
        mod.set_axon_ntff_profile_hook = lambda h: None
        sys.modules["antenv.axon_hooks"] = mod
        antenv.axon_hooks = mod
    except Exception:
        pass


_ensure_ntff_hook()

import os

R, F, IN, OUT = 4096, 32, 256, 512
N_CORES = 8
TOKENS = R * F                   # 131072
TPC = TOKENS // N_CORES          # 16384
KC = IN // 128                   # 2
BLK = 1024
NBLK = TPC // BLK                # 16
SUB = BLK // 128                 # 8
EPS = 1e-5
NEG_SLOPE = 0.01
BF16 = mybir.dt.bfloat16
F32 = mybir.dt.float32
AF = mybir.ActivationFunctionType
ALU = mybir.AluOpType

# normalize engine split: DVE norms per block (rest on ACT).
# DVE fixed load (bn_stats+aggr) >> ACT fixed (prelu), so ACT takes most.
_NDVE_EVEN = int(os.environ.get("KV3_NDVE_EVEN", "5"))
_NDVE_ODD = int(os.environ.get("KV3_NDVE_ODD", "4"))
_NDVE_TAIL = int(os.environ.get("KV3_NDVE_TAIL", "5"))  # blocks 14,15


def _n_dve(b):
    if b >= NBLK - 2:
        return _NDVE_TAIL
    return _NDVE_ODD if b % 2 else _NDVE_EVEN


_compiled = {}


def _build_nc():
    nc = bacc.Bacc(None)
    xT = nc.declare_dram_parameter("xT", [128, NBLK, KC, BLK], BF16, isOutput=False)
    w = nc.declare_dram_parameter("w", [KC, 128, OUT], BF16, isOutput=False)
    y = nc.declare_dram_parameter("y", [128, NBLK, SUB, OUT], BF16, isOutput=True)

    with tile.TileContext(nc) as tc:
        with (
            tc.tile_pool(name="singles", bufs=1) as singles,
            tc.tile_pool(name="xpool", bufs=4) as xpool,
            tc.tile_pool(name="ypool", bufs=4) as ypool,
            tc.tile_pool(name="opool", bufs=4) as opool,
            tc.tile_pool(name="stats", bufs=3) as stats_pool,
            tc.tile_pool(name="psum", bufs=2, space="PSUM") as psum,
        ):
            wdum = singles.tile([128, 128], BF16)
            nc.gpsimd.memset(wdum, 0.01)
            w_sb = singles.tile([128, KC, OUT], BF16)
            eps_sb = singles.tile([128, 1], F32)
            nc.vector.memset(eps_sb, EPS)
            # act-table prefetch: touch all three functions on a 1-elem tile
            # so the 1.3us table load happens during the x0 DMA, not before
            # the first real Prelu.
            tdum = singles.tile([128, 1], F32)
            nc.scalar.activation(tdum, eps_sb, AF.Prelu, alpha=NEG_SLOPE)
            nc.scalar.activation(tdum, eps_sb, AF.Sqrt)
            nc.scalar.activation(tdum, eps_sb, AF.Identity)

            def emit_norms(b, y_sb, mu, rstd, nmr):
                o_sb = opool.tile([128, SUB, OUT], BF16, name="o_sb")
                nd = _n_dve(b)
                for i in range(nd):
                    nc.vector.tensor_scalar(
                        o_sb[:, i, :], y_sb[:, i, :],
                        scalar1=mu[:, i:i + 1],
                        scalar2=rstd[:, i:i + 1],
                        op0=ALU.subtract, op1=ALU.mult,
                    )
                for i in range(nd, SUB):
                    nc.scalar.activation(
                        o_sb[:, i, :], y_sb[:, i, :], AF.Identity,
                        bias=nmr[:, i:i + 1],
                        scale=rstd[:, i:i + 1],
                    )
                if b == NBLK - 1:
                    for i in range(1, SUB, 2):
                        nc.sync.dma_start(
                            out=y[:, b, i - 1:i + 1], in_=o_sb[:, i - 1:i + 1])
                else:
                    nc.sync.dma_start(out=y[:, b, 0:4], in_=o_sb[:, 0:4])
                    nc.sync.dma_start(out=y[:, b, 4:8], in_=o_sb[:, 4:8])

            prev = None
            for b in range(NBLK):
                x_sb = xpool.tile([128, KC, BLK], BF16, name="x_sb")
                if b == 0:
                    # split x0 so the first k-chunk lands ~0.7us earlier and
                    # block 0's first matmuls start sooner
                    nc.sync.dma_start(out=x_sb[:, 0], in_=xT[:, b, 0])
                    nc.sync.dma_start(out=x_sb[:, 1], in_=xT[:, b, 1])
                else:
                    nc.sync.dma_start(out=x_sb, in_=xT[:, b])
                if b == 0:
                    for c in range(KC):
                        nc.sync.dma_start(out=w_sb[:, c, :], in_=w[c])

                y_sb = ypool.tile([128, SUB, OUT], BF16, name="y_sb")
                st = stats_pool.tile([128, 6, SUB], F32, name="st")

                for g in range(2):
                    ps = psum.tile([128, 4, OUT], F32, name="ps")
                    if b == 0 and g == 0:
                        # p-state warmup: keep PE busy from t~0.5us so the
                        # 0.65/1.2GHz ramp is spent on throwaway work.
                        for _ in range(8):
                            nc.tensor.matmul(
                                ps[:, 0, 0:128], lhsT=wdum, rhs=wdum,
                                start=True, stop=True,
                            )
                    for j in range(4):
                        i = g * 4 + j
                        nc.tensor.matmul(
                            ps[:, j, :], lhsT=x_sb[:, 0, bass.ts(i, 128)],
                            rhs=w_sb[:, 0, :], start=True, stop=False,
                        )
                        nc.tensor.matmul(
                            ps[:, j, :], lhsT=x_sb[:, 1, bass.ts(i, 128)],
                            rhs=w_sb[:, 1, :], start=False, stop=True,
                        )
                    if b == 0 and g == 0:
                        nc.scalar.activation(
                            y_sb[:, 0:2, :], ps[:, 0:2, :], AF.Prelu,
                            alpha=NEG_SLOPE,
                        )
                        nc.scalar.activation(
                            y_sb[:, 2:4, :], ps[:, 2:4, :], AF.Prelu,
                            alpha=NEG_SLOPE,
                        )
                    else:
                        nc.scalar.activation(
                            y_sb[:, g * 4:(g + 1) * 4, :], ps, AF.Prelu,
                            alpha=NEG_SLOPE,
                        )
                    for j in range(4):
                        i = g * 4 + j
                        nc.vector.bn_stats(st[:, :, i], y_sb[:, i, :])

                if prev is not None:
                    emit_norms(*prev)

                # combine even/odd bn_stats moments, batched over contiguous
                # [128, SUB] planes (replaces 8 bn_aggr with 7 cheap DVE ops):
                # mu = (me+mo)/2 ; var = (M2e+M2o)/512 + ((me-mo)/2)^2
                me, m2e = st[:, 1, :], st[:, 2, :]
                mo, m2o = st[:, 4, :], st[:, 5, :]
                sm = stats_pool.tile([128, SUB], F32, name="sm")
                dm = stats_pool.tile([128, SUB], F32, name="dm")
                dmh = stats_pool.tile([128, SUB], F32, name="dmh")
                dmq = stats_pool.tile([128, SUB], F32, name="dmq")
                sv = stats_pool.tile([128, SUB], F32, name="sv")
                mu = stats_pool.tile([128, SUB], F32, name="mu")
                var = stats_pool.tile([128, SUB], F32, name="var")
                nc.vector.tensor_tensor(sm, me, mo, ALU.add)
                nc.vector.tensor_scalar_mul(mu, sm, 0.5)
                nc.vector.tensor_tensor(dm, me, mo, ALU.subtract)
                nc.vector.tensor_scalar_mul(dmh, dm, 0.5)
                nc.vector.tensor_tensor(dmq, dmh, dmh, ALU.mult)
                nc.vector.tensor_tensor(sv, m2e, m2o, ALU.add)
                nc.vector.scalar_tensor_tensor(
                    var, sv, 1.0 / OUT, dmq, op0=ALU.mult, op1=ALU.add)

                std = stats_pool.tile([128, SUB], F32, name="std")
                nc.scalar.activation(std, var, AF.Sqrt, bias=eps_sb)
                rstd = stats_pool.tile([128, SUB], F32, name="rstd")
                nc.vector.reciprocal(rstd, std)
                # bias for ACT-normalized subtiles: -mean*rstd (one stt op)
                nmr = stats_pool.tile([128, SUB], F32, name="nmr")
                nc.vector.scalar_tensor_tensor(
                    nmr, mu, -1.0, rstd, op0=ALU.mult, op1=ALU.mult)


                prev = (b, y_sb, mu, rstd, nmr)
            emit_norms(*prev)
    nc.finalize()
    return nc


def _get_nc():
    if "nc" not in _compiled:
        _compiled["nc"] = _build_nc()
    return _compiled["nc"]


def _in_maps(x, W_v, W_r):
    x = np.asarray(x, dtype=np.float32)
    W = (np.asarray(W_v, dtype=np.float32).reshape(IN, OUT)
         + np.asarray(W_r, dtype=np.float32))
    w_dev = np.ascontiguousarray(
        W.reshape(KC, 128, OUT).astype(ml_dtypes.bfloat16))

    xs = x.reshape(TOKENS, IN)
    in_maps = []
    for c in range(N_CORES):
        shard = xs[c * TPC:(c + 1) * TPC]                    # [TPC, IN]
        xt = np.ascontiguousarray(
            shard.reshape(NBLK, BLK, KC, 128).transpose(3, 0, 2, 1)
            .astype(ml_dtypes.bfloat16))                     # [128,NBLK,KC,BLK]
        in_maps.append({"xT": xt, "w": w_dev})
    return in_maps


def _gather(res):
    outs = []
    for c in range(N_CORES):
        yd = np.asarray(res.results[c]["y"])                 # [128,NBLK,SUB,OUT]
        outs.append(yd.astype(np.float32).transpose(1, 2, 0, 3).reshape(TPC, OUT))
    return np.concatenate(outs, axis=0).reshape(R, F, OUT)


def kernel(x, W_q, W_k, W_v, W_r, ln_gamma, ln_beta):
    nc = _get_nc()
    in_maps = _in_maps(x, W_v, W_r)
    res = run_bass_kernel_spmd(nc, in_maps, list(range(N_CORES)))
    out = _gather(res)

    gamma = np.asarray(ln_gamma, dtype=np.float32)
    beta = np.asarray(ln_beta, dtype=np.float32)
    if not (np.all(gamma == 1.0) and np.all(beta == 0.0)):
        out = out * gamma + beta
    return out.astype(np.float32)
